# revision 1
# baseline (speedup 1.0000x reference)
"""Trainium2 Bass kernel for nn_InterleavedHiddenMarkovChain_47261820125822.

Math: in the reference, the dense (N,N) score matrix M (N = S*S*K = 4608)
is -inf except where the full state tuple of x_old equals x_new's (the
`same` mask compares all K components), so each column has exactly K=2
finite entries and the scan collapses exactly.  With
g[c,s,y] = choice_l[c] + trans_l[c,s,s] + emis_l[c,s,y]:

    beta_0[s0,s1] = prior_l[0,s0] + prior_l[1,s1] + LSE_c(choice_l)
    beta_t = beta_{t-1} + h_t,  h_t[s0,s1] = LSE(g[0,s0,y_t], g[1,s1,y_t])
    answer = LSE_{s0,s1} beta_T

This is bitwise-equal math to the dense scan (the -inf entries contribute
exact zeros to each logsumexp).  Using LSE(a,b) = b + log1p(exp(a-b))
(|a-b| < 40 here, so no overflow), sum_t splits into sum_t G1[s1,t]
(separable; one ones-matmul) + sum_t log1p(exp(G0[s0,t]-G1[s1,t])) — a
single (T=64 partitions) x (48*48 free) fused elementwise pass.

Sharding across the 8 cores: the collapsed problem is ~150K flops, far
below per-core fixed overheads, so the sharding-hint's row-sharded psum
scheme would be pure loss.  We replicate: all 8 cores run the identical
NEFF SPMD (the hint is advisory; "distribute as you see fit"), and the
host takes core 0's scalar.  All floating-point work happens on-device;
the host only reshapes inputs, builds the one-hot of ys (index prep),
and constant tensors (identity / ones).
"""

import numpy as np

import concourse.bass as bass
import concourse.bacc as bacc
import concourse.mybir as mybir
from concourse import tile
from concourse.bass_utils import run_bass_kernel_spmd

F32 = mybir.dt.float32
AF = mybir.ActivationFunctionType
AX = mybir.AxisListType
OP = mybir.AluOpType

K, S, A, T = 2, 48, 64, 64
CS = K * S          # 96 (c,s) rows
N2 = S * S          # 2304
N_CORES = 8

_CACHED_NC = None


def _build_nc():
    nc = bacc.Bacc("TRN2", target_bir_lowering=False, debug=False)

    tr = nc.dram_tensor("trans", [CS, S], F32, kind="ExternalInput")
    em = nc.dram_tensor("emis", [CS, A], F32, kind="ExternalInput")
    pr = nc.dram_tensor("prior", [K, S], F32, kind="ExternalInput")
    ch = nc.dram_tensor("choice", [1, K], F32, kind="ExternalInput")
    yoh = nc.dram_tensor("yoh", [A, T], F32, kind="ExternalInput")
    id96 = nc.dram_tensor("id96", [CS, CS], F32, kind="ExternalInput")
    ones64 = nc.dram_tensor("ones64", [T, 1], F32, kind="ExternalInput")
    out_d = nc.dram_tensor("out", [1, 1], F32, kind="ExternalOutput")

    with tile.TileContext(nc) as tc:
        with (
            tc.tile_pool(name="sb", bufs=1) as sb,
            tc.tile_pool(name="ps", bufs=1, space="PSUM") as ps,
        ):
            def load(name, dram, shape):
                t = sb.tile(shape, F32, tag=name)
                nc.sync.dma_start(t[:], dram[:, :])
                return t

            TT = load("TT", tr, [CS, S])
            EM = load("EM", em, [CS, A])
            CH = load("CH", ch, [1, K])
            # prior rows as separate partition-0 tiles (engine APs may only
            # start at partitions {0,32,64,96})
            PR0 = sb.tile([1, S], F32, tag="PR0")
            nc.sync.dma_start(PR0[:], pr[0:1, :])
            PR1 = sb.tile([1, S], F32, tag="PR1")
            nc.sync.dma_start(PR1[:], pr[1:2, :])
            YOH = load("YOH", yoh, [A, T])
            ID = load("ID", id96, [CS, CS])
            ON = load("ON", ones64, [T, 1])
            # diagonal transition[c,s,s]: per c-block a stride-(S+1) walk
            DG = sb.tile([CS, 1], F32, tag="DG")
            nc.sync.dma_start(
                DG[:], bass.AP(tr, 0, [[S * S, K], [S + 1, S], [1, 1]]))

            def row_lse(x_ap, P, W, name):
                """per-partition logsumexp over the free axis -> (P,1)"""
                nm = sb.tile([P, 1], F32, tag=f"nm_{name}")
                nc.vector.tensor_reduce(nm[:], x_ap, axis=AX.X, op=OP.max,
                                        negate=True)
                e = sb.tile([P, W], F32, tag=f"e_{name}")
                nc.scalar.activation(e[:], x_ap, AF.Exp, bias=nm[:])
                s = sb.tile([P, 1], F32, tag=f"s_{name}")
                nc.vector.tensor_reduce(s[:], e[:], axis=AX.X, op=OP.add)
                l = sb.tile([P, 1], F32, tag=f"l_{name}")
                nc.scalar.activation(l[:], s[:], AF.Ln)
                lse = sb.tile([P, 1], F32, tag=f"lse_{name}")
                nc.vector.tensor_sub(lse[:], l[:], nm[:])
                return lse

            lseT = row_lse(TT[:], CS, S, "T")
            lseE = row_lse(EM[:], CS, A, "E")

            lseC = row_lse(CH[:], 1, K, "C")
            CHL = sb.tile([1, K], F32, tag="CHL")
            nc.vector.tensor_scalar_sub(CHL[:], CH[:], lseC[:])
            cL = row_lse(CHL[:], 1, K, "C2")      # LSE_c choice_l  (~0)

            lseP0 = row_lse(PR0[:], 1, S, "P0")
            PRL0 = sb.tile([1, S], F32, tag="PRL0")
            nc.vector.tensor_scalar_sub(PRL0[:], PR0[:], lseP0[:])
            lseP1 = row_lse(PR1[:], 1, S, "P1")
            PRL1 = sb.tile([1, S], F32, tag="PRL1")
            nc.vector.tensor_scalar_sub(PRL1[:], PR1[:], lseP1[:])

            # choice_l along free as (1, 96) for the accumulate-matmul:
            # CROW[0, c*S+s] = choice_l[c]
            CROW = sb.tile([1, CS], F32, tag="CROW")
            nc.vector.tensor_copy(
                CROW[:].rearrange("p (c s) -> p c s", c=K, s=S),
                CHL[:].unsqueeze(2).broadcast_to([1, K, S]))
            ONR = sb.tile([1, T], F32, tag="ONR")
            nc.vector.memset(ONR[:], 1.0)

            # per-(c,s) additive term: diag - lseT - lseE
            t0 = sb.tile([CS, 1], F32, tag="t0")
            nc.vector.tensor_sub(t0[:], DG[:], lseT[:])
            PCOL = sb.tile([CS, 1], F32, tag="PCOL")
            nc.vector.tensor_sub(PCOL[:], t0[:], lseE[:])

            # G_full[(c,s), a] = emission + per-row constant
            GF = sb.tile([CS, A], F32, tag="GF")
            nc.vector.tensor_scalar_add(GF[:], EM[:], PCOL[:])

            # transpose to (a, (c,s)), then gather columns by ys via the
            # one-hot matmul; choice_l rides in as a rank-1 accumulate
            GFT_p = ps.tile([A, CS], F32, tag="ps_small")
            nc.tensor.transpose(GFT_p[:], GF[:], ID[:])
            GFT = sb.tile([A, CS], F32, tag="GFT")
            nc.vector.tensor_copy(GFT[:], GFT_p[:])
            GT_p = ps.tile([T, CS], F32, tag="ps_small")
            nc.tensor.matmul(GT_p[:], YOH[:], GFT[:], start=True, stop=False)
            nc.tensor.matmul(GT_p[:], ONR[:], CROW[:], start=False, stop=True)
            GTs = sb.tile([T, CS], F32, tag="GTs")
            nc.vector.tensor_copy(GTs[:], GT_p[:])

            # d[t, s0, s1] = G0[t,s0] - G1[t,s1]  via stride-0 broadcasts
            u0 = GTs[:, 0:S]
            u1 = GTs[:, S:CS]
            u0b = u0.unsqueeze(2).broadcast_to([T, S, S])
            u1b = u1.unsqueeze(1).broadcast_to([T, S, S])
            D = sb.tile([T, S, S], F32, tag="D")
            nc.vector.tensor_sub(D[:], u0b, u1b)
            Df = D[:].rearrange("p a b -> p (a b)")
            EX = sb.tile([T, N2], F32, tag="EX")
            nc.scalar.activation(EX[:], Df, AF.Exp)
            SP = sb.tile([T, N2], F32, tag="SP")
            nc.scalar.activation(SP[:], EX[:], AF.Ln, bias=1.0)

            # NL[s0,s1] = sum_t SP  (contract partition dim with ones)
            NL_p = ps.tile([1, N2], F32, tag="NL_p")
            for j0 in range(0, N2, 512):
                w = min(512, N2 - j0)
                nc.tensor.matmul(NL_p[:, j0:j0 + w], ON[:], SP[:, j0:j0 + w])
            R1_p = ps.tile([1, S], F32, tag="ps_small")
            nc.tensor.matmul(R1_p[:], ON[:], u1)

            NLs = sb.tile([1, N2], F32, tag="NLs")
            nc.vector.tensor_copy(NLs[:], NL_p[:])
            R1s = sb.tile([1, S], F32, tag="R1s")
            nc.vector.tensor_copy(R1s[:], R1_p[:])

            # total[s0,s1] = NL + R1[s1] + prior_l[0,s0] + prior_l[1,s1] + cL
            T1 = sb.tile([1, S, S], F32, tag="T1")
            nc.vector.tensor_add(
                T1[:], NLs[:].rearrange("p (a b) -> p a b", a=S, b=S),
                R1s[:].unsqueeze(1).broadcast_to([1, S, S]))
            T2 = sb.tile([1, S, S], F32, tag="T2")
            nc.vector.tensor_add(
                T2[:], T1[:], PRL0[:].unsqueeze(2).broadcast_to([1, S, S]))
            T3 = sb.tile([1, S, S], F32, tag="T3")
            nc.vector.tensor_add(
                T3[:], T2[:], PRL1[:].unsqueeze(1).broadcast_to([1, S, S]))
            T4 = sb.tile([1, N2], F32, tag="T4")
            nc.vector.tensor_scalar_add(
                T4[:], T3[:].rearrange("p a b -> p (a b)"), cL[:])

            # final logsumexp over all 2304 entries
            fin = row_lse(T4[:], 1, N2, "F")
            nc.sync.dma_start(out_d[:, :], fin[:])

    nc.compile()
    return nc


def _host_inputs(ys, transition, emission, choice, prior):
    ys = np.asarray(ys).astype(np.int64)
    yoh = (ys[None, :] == np.arange(A)[:, None]).astype(np.float32)
    return {
        "trans": np.ascontiguousarray(
            np.asarray(transition, np.float32).reshape(CS, S)),
        "emis": np.ascontiguousarray(
            np.asarray(emission, np.float32).reshape(CS, A)),
        "prior": np.ascontiguousarray(np.asarray(prior, np.float32)),
        "choice": np.asarray(choice, np.float32).reshape(1, K),
        "yoh": yoh,
        "id96": np.eye(CS, dtype=np.float32),
        "ones64": np.ones((T, 1), np.float32),
    }


def kernel(ys, transition, emission, choice, prior):
    global _CACHED_NC
    if _CACHED_NC is None:
        _CACHED_NC = _build_nc()
    in_map = _host_inputs(ys, transition, emission, choice, prior)
    in_maps = [dict(in_map) for _ in range(N_CORES)]
    res = run_bass_kernel_spmd(_CACHED_NC, in_maps,
                               core_ids=list(range(N_CORES)))
    return np.float32(res.results[0]["out"][0, 0]).reshape(())
